# revision 29
# baseline (speedup 1.0000x reference)
"""AttentionPooling GNN kernel for 8 Trainium2 NeuronCores.

Strategy (v2)
-------------
Graph-parallel sharding: 128 graphs -> 16 graphs per core; each core gets its
graphs' nodes and (re-grouped) edges.  Host does index preprocessing only
(edge permutation, packing, weight folding); all FLOPs on device.

Device algorithm (per core):
 1. Edge stream in fp8e4m3, sorted by destination node, padded to 4-edge
    slots inside 128-edge chunks (32 slots, no pad slot).  A shared
    triangular stationary (TriU4, fp8) turns chunk matmuls into slot-granular
    prefix sums P4 accumulated in PSUM fp32; 4 col-groups packed via
    tile_position.  ACT copies P4 to a per-batch SBUF table in bf16 with each
    (chunk,slot) row duplicated to 256B so it is gatherable.
 2. Nodes are processed in *packing order* (per-128-block, desc-degree).
    One SBUF->SBUF transposed dma_gather per batch-half pulls each node's
    run-end prefix row feature-major: phi[feat, node].  Because runs are
    back-to-back inside a chunk, the run-start prefix of node i is the
    run-end prefix of node i-1 (or exact zero at chunk starts), so
    A = phi_i - m_i * phi_{i-1} (m is a host-shipped 0/1 row-replicated
    mask) and meanA = A * inv_deg -- 3 DVE ops writing the wb2 stationary
    augT directly.  No lo-gather, no PE transposes.
 3. The linear chain (node proj + edge proj + v-proj + attention scores) is
    folded host-side into one [194, 260] matrix applied as two matmuls per
    128-node chunk: [h(128)] @ wb1 + [meanA(64)|has_edge|1] @ wb2
    -> [v(256) | scores(4)] in PSUM.
 4. w = exp(scores) (ACT); pr = [w*v | w] (DVE); pooling = matmul with the
    per-chunk one-hot graph-membership matrix accumulated in PSUM
    -> [16, 260] segment sums.
 5. pooled = U/denom; out = pooled @ out_w.T + out_b (fp32).
"""
import sys

sys.path.insert(0, "/opt/trn_rl_repo")

import numpy as np

NUM_HEADS = 4
G_TOTAL = 128
CORES = 8
GL = G_TOTAL // CORES       # graphs per core
P = 128                     # partitions
SLOT = 4                    # edges per slot
SPC = 32                    # slots per chunk (128 edges)
CPG = 32                    # chunks per group (4096 edges)
GROUP_E = P * CPG           # 4096 edges per group
KB_LIST = [8, 8, 8, 7, 7, 6, 5, 3]   # node chunks per gather batch
S_BATCH = len(KB_LIST)
KB0 = [0]
for _kb in KB_LIST:
    KB0.append(KB0[-1] + _kb)
NKB = KB0[-1]                        # 52 node chunks
NC_NODES = NKB * P                   # 6656 padded nodes per core
DENSE_LAG = 1
ES_FP8 = True
DEBUG_DUMP = False
GATHER_SINGLE_PACKET = True
GATHER_QUEUES = 4


# ----------------------------------------------------------------- host prep
def _pack_core(deg):
    """Pack node runs (padded to 4-edge slots) into 32-slot chunks in
    descending-degree order per 128-node block; no pad slots, runs
    back-to-back, no run straddles a chunk, each block starts a fresh chunk.

    Returns (perm [NL] packing order -> original local node,
             s0 [NL] global start slot (packing order, -1 for deg 0),
             m [NL] 1 if run does not start a chunk (packing order),
             blk_chunks [NB]).
    """
    NL = len(deg)
    r = (deg + SLOT - 1) // SLOT
    assert r.max(initial=0) <= SPC, "node degree > 128 unsupported"
    nblocks = (NL + P - 1) // P
    perm = np.zeros(NL, np.int64)
    s0 = np.full(NL, -1, np.int64)
    m = np.zeros(NL, np.int64)
    blk_chunks = np.zeros(nblocks, np.int64)
    cur = 0  # global slot cursor
    pos = 0
    for kb in range(nblocks):
        if cur % SPC:
            cur = (cur // SPC + 1) * SPC
        start_chunk = cur // SPC
        blk_nodes = list(range(kb * P, min((kb + 1) * P, NL)))
        blk_nodes.sort(key=lambda n: (-r[n], n))
        for n in blk_nodes:
            rn = int(r[n])
            perm[pos] = n
            if rn:
                if cur % SPC + rn > SPC:
                    cur = (cur // SPC + 1) * SPC
                s0[pos] = cur
                m[pos] = 1 if cur % SPC else 0
                cur += rn
            pos += 1
        blk_chunks[kb] = (cur + SPC - 1) // SPC - start_chunk
        cur = (cur + SPC - 1) // SPC * SPC
    return perm, s0, m, blk_chunks


def _prep(h, edge_index, edge_attr, batch):
    """Shard + pack. Returns per-core dict of host arrays + shared config."""
    row = np.asarray(edge_index[0], np.int64)
    batch = np.asarray(batch, np.int64)
    gstart = np.searchsorted(batch, np.arange(G_TOTAL + 1))
    order = np.argsort(row, kind="stable")
    row_s = row[order]

    cores = []
    for c in range(CORES):
        n0, n1 = int(gstart[GL * c]), int(gstart[GL * (c + 1)])
        NL = n1 - n0
        assert NL <= NC_NODES, (NL, NC_NODES)
        e0, e1 = np.searchsorted(row_s, [n0, n1])
        eord = order[e0:e1]
        lrow = row_s[e0:e1] - n0
        deg = np.bincount(lrow, minlength=NL)
        perm, s0, m, blk_chunks = _pack_core(deg)
        nb = len(blk_chunks)
        need = np.zeros(S_BATCH, np.int64)
        for s in range(S_BATCH):
            ch = blk_chunks[KB0[s]:min(KB0[s + 1], nb)].sum()
            need[s] = ch // CPG + 1          # >=1 spare chunk (zero row)
        cores.append(dict(n0=n0, n1=n1, NL=NL, eord=eord, lrow=lrow, deg=deg,
                          perm=perm, s0=s0, m=m, blk_chunks=blk_chunks,
                          need=need))
    B_S = [max(int(st["need"][s]) for st in cores) for s in range(S_BATCH)]
    G0 = [0]
    for b in B_S:
        G0.append(G0[-1] + b)
    NGROUPS = G0[-1]
    E_PAD = NGROUPS * GROUP_E
    assert max(B_S) * 1024 <= 32767

    for c, st in enumerate(cores):
        deg, perm, s0, m, blk_chunks = (st["deg"], st["perm"], st["s0"],
                                        st["m"], st["blk_chunks"])
        NL = st["NL"]
        nb = len(blk_chunks)
        # shift block-sequential slots to batch-aligned global slots
        kb_batch = np.searchsorted(np.asarray(KB0[1:]), np.arange(nb),
                                   side="right")
        blk_chunk0 = np.zeros(nb, np.int64)
        used = np.zeros(S_BATCH, np.int64)
        cur_chunk = 0
        for kb in range(nb):
            s = int(kb_batch[kb])
            if kb == KB0[s]:
                cur_chunk = G0[s] * CPG
            blk_chunk0[kb] = cur_chunk
            cur_chunk += blk_chunks[kb]
            used[s] = cur_chunk - G0[s] * CPG
            assert cur_chunk < G0[s + 1] * CPG  # strict: spare chunk exists
        orig_start = np.concatenate([[0], np.cumsum(blk_chunks)])[:-1]
        shift = (blk_chunk0 - orig_start) * SPC
        pos_blk = np.arange(NL) // P
        s0g = np.where(s0 >= 0, s0 + shift[pos_blk], -1)

        # edge stream positions: edges of node perm[i] occupy slots starting
        # at s0g[i]; swizzle to partition-major DRAM layout
        first_edge = np.concatenate([[0], np.cumsum(deg)])[:-1]
        degp = deg[perm]                       # degree in packing order
        has = degp > 0
        epos_base = np.repeat(SLOT * s0g[has], degp[has])
        within = np.concatenate(
            [np.arange(d) for d in degp[has]]) if has.any() else np.zeros(0, np.int64)
        src = np.repeat(first_edge[perm[has]], degp[has]) + within
        epos = epos_base + within
        assert epos.max(initial=-1) < E_PAD
        eg = epos // GROUP_E
        ep = epos % P
        ec = (epos % GROUP_E) // P
        st["epos"] = eg * GROUP_E + ep * CPG + ec
        st["esrc"] = st["eord"][src]

        # per-node (packing order) hi-row local table index + m flag
        def slot_to_idx(sl_g):
            chunk = sl_g // SPC
            sl = sl_g % SPC
            g = chunk // CPG
            s = np.searchsorted(np.asarray(G0[1:]), g, side="right")
            gg = g - np.asarray(G0)[s]
            ch = chunk % CPG
            return (gg * 8 + ch % 8) * P + 32 * (ch // 8) + sl

        hi = np.where(s0g >= 0, s0g + (degp + SLOT - 1) // SLOT - 1, 0)
        hi_idx = np.where(s0g >= 0, slot_to_idx(np.maximum(hi, 0)), 0)
        # zero-row (first unused chunk) per batch
        pos_batch = kb_batch[pos_blk]
        zero_idx = np.zeros(S_BATCH, np.int64)
        for s in range(S_BATCH):
            cu = int(used[s])
            zero_idx[s] = (cu // CPG * 8 + cu % CPG % 8) * P + 32 * (cu % CPG // 8)
        hi_idx = np.where(s0g >= 0, hi_idx, zero_idx[pos_batch])
        mm = np.where(s0g >= 0, m, 0)
        # pad nodes to NC_NODES (all use last batch's zero row, m=0)
        hi_row = np.concatenate([hi_idx, np.full(NC_NODES - NL,
                                                 zero_idx[S_BATCH - 1])])
        mrow = np.concatenate([mm, np.zeros(NC_NODES - NL, np.int64)])
        assert hi_row.max() < max(B_S) * 1024
        st.update(hi_row=hi_row.astype(np.int64), mrow=mrow)
    cfg = dict(B_S=B_S, G0=G0, NGROUPS=NGROUPS, E_PAD=E_PAD)
    return cores, cfg


def _wrap_idx(a, npart_rep=8):
    """[M] -> [128, M//16] int16, F-wrapped 16-row block replicated 8x."""
    mm = a.reshape(-1, 16).T.astype(np.int16)
    return np.tile(mm, (npart_rep, 1))


def _fold_weights(node_w, node_b, edge_w, edge_b, query, in_w, in_b):
    D = query.shape[-1]
    dh = D // NUM_HEADS
    wq, wk, wv = in_w[:D], in_w[D:2 * D], in_w[2 * D:]
    bq, bk, bv = in_b[:D], in_b[D:2 * D], in_b[2 * D:]
    q = (query[0] @ wq.T + bq).reshape(NUM_HEADS, dh)
    s_w = np.einsum("hj,hjd->dh", q, wk.reshape(NUM_HEADS, dh, D)) / np.sqrt(dh)
    s_b = np.einsum("hj,hj->h", q, bk.reshape(NUM_HEADS, dh)) / np.sqrt(dh)
    A1 = np.concatenate([node_w.T, edge_w.T, edge_b[None, :], node_b[None, :]],
                        axis=0)
    M2 = np.concatenate([wv.T, s_w], axis=1)
    Wbig = A1 @ M2                                     # [194, 260]
    Wbig[-1, :256] += bv
    Wbig[-1, 256:] += s_b
    return Wbig.astype(np.float32)


# ------------------------------------------------------- numpy device model
def _numpy_device_model(cores, cfg, h, edge_attr, batch, Wbig, out_w, out_b):
    """Bit-approximate emulation of the device program (fp8/bf16 rounding
    where the device rounds) — validates packing/indexing host-side."""
    import ml_dtypes
    bf = lambda x: x.astype(ml_dtypes.bfloat16).astype(np.float32)
    f8 = lambda x: x.astype(ml_dtypes.float8_e4m3).astype(np.float32)
    B_S, G0, E_PAD = cfg["B_S"], cfg["G0"], cfg["E_PAD"]
    outs = []
    for c, st in enumerate(cores):
        NL = st["NL"]
        stream = np.zeros((E_PAD, 64), np.float32)
        stream[st["epos"]] = edge_attr[st["esrc"]]
        streamq = f8(stream) if ES_FP8 else bf(stream)
        # per-batch P4 tables, bf16 rows
        tabs = []
        for s in range(S_BATCH):
            tab = np.zeros((B_S[s] * 1024, 64), np.float32)
            for gg in range(B_S[s]):
                g = G0[s] + gg
                for ch in range(CPG):
                    # chunk edges: global positions g*4096 + p*32 + ch for p
                    cdat = streamq[g * GROUP_E + np.arange(P) * CPG + ch]
                    pre = np.add.reduceat(cdat, np.arange(0, P, SLOT), 0).cumsum(0)
                    idx = (gg * 8 + ch % 8) * P + 32 * (ch // 8) + np.arange(SPC)
                    tab[idx] = pre
            tabs.append(bf(tab))
        # gather phi in packing order
        kb_of_pos = np.searchsorted(np.asarray(KB0[1:]), np.arange(NC_NODES) // P,
                                    side="right")
        phi = np.zeros((NC_NODES + 1, 64), np.float32)
        for i in range(NC_NODES):
            phi[i + 1] = tabs[kb_of_pos[i]][st["hi_row"][i]]
        mrep = st["mrow"].astype(np.float32)
        degp = np.zeros(NC_NODES, np.float32)
        degp[:NL] = st["deg"][st["perm"]]
        inv = 1.0 / np.maximum(degp, 1.0)
        t = bf(phi[:-1] * mrep[:, None])
        a = bf(phi[1:] - t)
        meanA = bf(a * inv[:, None])
        hase = (degp > 0).astype(np.float32)
        hpad = np.zeros((NC_NODES, 128), np.float32)
        hpad[:NL] = h[st["n0"]:st["n1"]][st["perm"]]
        xaug = np.concatenate([bf(hpad), meanA, bf(hase[:, None]),
                               np.ones((NC_NODES, 1), np.float32)], 1)
        vs = bf(xaug) @ bf(Wbig)
        v, sc = vs[:, :256], vs[:, 256:]
        w = np.exp(sc)
        bl = np.full(NC_NODES, -1, np.int64)
        bl[:NL] = (batch[st["n0"]:st["n1"]] - GL * c)[st["perm"]]
        onehot = (bl[:, None] == np.arange(GL)[None, :]).astype(np.float32)
        wv4 = np.concatenate(
            [bf(w[:, :, None] * v.reshape(-1, 4, 64)).reshape(-1, 256), bf(w)], 1)
        U = bf(onehot).T @ wv4
        den = np.maximum(U[:, 256:], 1e-30)
        pooled = U[:, :256].reshape(GL, 4, 64) / den[:, :, None]
        o = pooled.reshape(GL, 256) @ out_w.T + out_b
        outs.append(o)
    return np.concatenate(outs).reshape(G_TOTAL, 1, 256)


# ------------------------------------------------------------- bass program
def _build_program(cfg):
    import concourse.bacc as bacc
    import concourse.mybir as mybir
    import concourse.tile as tile

    F32 = mybir.dt.float32
    BF16 = mybir.dt.bfloat16
    FP8 = mybir.dt.float8e4 if ES_FP8 else mybir.dt.bfloat16
    I16 = mybir.dt.int16
    AF = mybir.ActivationFunctionType
    B_S, G0, NGROUPS, E_PAD = cfg["B_S"], cfg["G0"], cfg["NGROUPS"], cfg["E_PAD"]

    nc = bacc.Bacc("TRN2", num_devices=CORES, num_swdge_queues=4)
    es_d = nc.dram_tensor("es", [E_PAD, 64], FP8, kind="ExternalInput")
    h_d = nc.dram_tensor("h", [P, NKB, 128], BF16, kind="ExternalInput")
    bid_d = nc.dram_tensor("bid", [P, NKB], F32, kind="ExternalInput")
    hi_d = nc.dram_tensor("hi", [P, NC_NODES // 16], I16, kind="ExternalInput")
    inv_d = nc.dram_tensor("inv", [64, NC_NODES], BF16, kind="ExternalInput")
    mrep_d = nc.dram_tensor("mrep", [64, NC_NODES], BF16, kind="ExternalInput")
    hone_d = nc.dram_tensor("hone", [2, NKB, 128], BF16, kind="ExternalInput")
    tri_d = nc.dram_tensor("tri", [P, SPC], FP8, kind="ExternalInput")
    idtf_d = nc.dram_tensor("idtf", [P, P], F32, kind="ExternalInput")
    iota_d = nc.dram_tensor("iota", [P, GL], F32, kind="ExternalInput")
    wb1_d = nc.dram_tensor("wb1", [128, 260], BF16, kind="ExternalInput")
    wb2_d = nc.dram_tensor("wb2", [66, 260], BF16, kind="ExternalInput")
    owt_d = nc.dram_tensor("owt", [256, 256], F32, kind="ExternalInput")
    ob_d = nc.dram_tensor("ob", [GL, 256], F32, kind="ExternalInput")
    y_d = nc.dram_tensor("y", [GL, 256], F32, kind="ExternalOutput")
    if DEBUG_DUMP:
        phid_d = nc.dram_tensor("phid", [P, NC_NODES + 16], BF16,
                                kind="ExternalOutput")
        tabd_d = nc.dram_tensor("tabd", [P, B_S[0] * 1024], BF16,
                                kind="ExternalOutput")
        augd_d = nc.dram_tensor("augd", [66, NKB * 128], BF16,
                                kind="ExternalOutput")

    with tile.TileContext(nc) as tc:
        with tc.tile_pool(name="const", bufs=1) as cp, \
             tc.tile_pool(name="sb", bufs=3) as sb, \
             tc.tile_pool(name="big", bufs=1) as bigp, \
             tc.tile_pool(name="ps", bufs=2, space="PSUM") as ps, \
             tc.tile_pool(name="pacc", bufs=1, space="PSUM") as pacc:

            trib = cp.tile([P, SPC], FP8, name="trib")
            nc.sync.dma_start(out=trib[:], in_=tri_d.ap()[:, :])
            idtf = cp.tile([P, P], F32, name="idtf")
            nc.sync.dma_start(out=idtf[:], in_=idtf_d.ap()[:, :])
            iot = cp.tile([P, GL], F32, name="iot")
            nc.sync.dma_start(out=iot[:], in_=iota_d.ap()[:, :])
            wb1 = cp.tile([128, 260], BF16, name="wb1")
            nc.sync.dma_start(out=wb1[:], in_=wb1_d.ap()[:, :])
            wb2 = cp.tile([66, 260], BF16, name="wb2")
            nc.sync.dma_start(out=wb2[:], in_=wb2_d.ap()[:, :])
            owt = cp.tile([P, 2, 256], F32, name="owt")
            nc.sync.dma_start(out=owt[:],
                              in_=owt_d.ap()[:, :].rearrange("(i p) f -> p i f", p=P))
            obt = cp.tile([GL, 256], F32, name="obt")
            nc.sync.dma_start(out=obt[:], in_=ob_d.ap()[:, :])
            hi_t = cp.tile([P, NC_NODES // 16], I16, name="hi_t")
            nc.sync.dma_start(out=hi_t[:], in_=hi_d.ap()[:, :])

            # big node-side loads deferred to group 2 (ACT HWDGE queue) so the
            # first es groups get the full HBM bandwidth
            hsb = bigp.tile([P, NKB, 128], BF16, name="hsb")
            bid = bigp.tile([P, NKB], F32, name="bid")
            invr = bigp.tile([64, NC_NODES], BF16, name="invr")
            mrep = bigp.tile([64, NC_NODES], BF16, name="mrep")
            augT = bigp.tile([66, NKB, 128], BF16, name="augT")

            def issue_node_loads():
                nc.scalar.dma_start(out=invr[:], in_=inv_d.ap()[:, :])
                nc.scalar.dma_start(out=mrep[:], in_=mrep_d.ap()[:, :])
                nc.scalar.dma_start(out=hsb[:], in_=h_d.ap()[:, :, :])
                nc.scalar.dma_start(out=bid[:], in_=bid_d.ap()[:, :])
                nc.scalar.dma_start(out=augT[64:66, :, :],
                                    in_=hone_d.ap()[:, :, :])
            memall = bigp.tile([P, NKB, GL], BF16, name="memall")
            # gather dst must stay 32B-aligned: zero column lives at 15,
            # gathered node i at column 16+i
            phi = bigp.tile([P, NC_NODES + 16], BF16, name="phi")
            nc.vector.memset(phi[:, 0:16], 0)
            tabs = [bigp.tile([P, B_S[s] * 8, 2, 64], BF16, name=f"tab{s}")
                    for s in range(S_BATCH)]

            pool_ps = pacc.tile([GL, 260], F32, name="pool_ps")

            def emit_meanA(s):
                k0, KBB = KB0[s], KB_LIST[s]
                n0c, n1c = k0 * P, (k0 + KBB) * P
                # meanA = (phi_i - m*phi_{i-1}) * inv  -> augT rows 0..63
                tmp = sb.tile([64, KBB * P], BF16, name="tmp", tag="tmp", bufs=2)
                nc.vector.tensor_tensor(out=tmp[:], in0=phi[0:64, 15 + n0c:15 + n1c],
                                        in1=mrep[:, n0c:n1c],
                                        op=mybir.AluOpType.mult)
                av = sb.tile([64, KBB * P], BF16, name="av", tag="av", bufs=2)
                nc.vector.tensor_tensor(out=av[:], in0=phi[0:64, 16 + n0c:16 + n1c],
                                        in1=tmp[:], op=mybir.AluOpType.subtract)
                nc.vector.tensor_tensor(
                    out=augT[0:64, k0:k0 + KBB, :].rearrange("p a b -> p (a b)"),
                    in0=av[:], in1=invr[:, n0c:n1c], op=mybir.AluOpType.mult)
                nc.vector.tensor_tensor(
                    out=memall[:, k0:k0 + KBB, :],
                    in0=iot[:].broadcast_to([P, GL, KBB]).rearrange("p g k -> p k g"),
                    in1=bid[:, k0:k0 + KBB].broadcast_to([P, KBB, GL]),
                    op=mybir.AluOpType.is_equal)

            pending_pool = []     # [(k, pr_tile)] single entry, lag-1

            def flush_pool(stop):
                if pending_pool:
                    k, pr = pending_pool.pop()
                    nc.tensor.matmul(out=pool_ps[:], lhsT=memall[:, k, :],
                                     rhs=pr[:], start=(k == 0), stop=stop)

            def emit_batch_dense(s):
                k0, KBB = KB0[s], KB_LIST[s]
                for k in range(k0, k0 + KBB):
                    vs = ps.tile([P, 260], F32, name="vs", tag="vs", bufs=3)
                    nc.tensor.matmul(out=vs[:], lhsT=hsb[:, k, :], rhs=wb1[:],
                                     start=True, stop=False)
                    nc.tensor.matmul(out=vs[:], lhsT=augT[:66, k, :], rhs=wb2[:],
                                     start=False, stop=True)
                    flush_pool(False)
                    wsb = sb.tile([P, 4], F32, name="wsb", tag="wsb", bufs=4)
                    nc.scalar.activation(out=wsb[:], in_=vs[:, 256:260], func=AF.Exp)
                    pr = sb.tile([P, 260], BF16, name="pr", tag="pr", bufs=4)
                    nc.vector.tensor_tensor(
                        out=pr[:, :256].rearrange("p (h f) -> p h f", h=NUM_HEADS),
                        in0=vs[:, :256].rearrange("p (h f) -> p h f", h=NUM_HEADS),
                        in1=wsb[:].broadcast_to([P, NUM_HEADS, 64]),
                        op=mybir.AluOpType.mult)
                    nc.vector.tensor_copy(out=pr[:, 256:260], in_=wsb[:])
                    pending_pool.append((k, pr))

            import bisect
            for g in range(NGROUPS):
                s = bisect.bisect_right(G0, g) - 1
                gg = g - G0[s]
                et = sb.tile([P, CPG, 64], FP8, name="et", tag="et", bufs=4)
                nc.sync.dma_start(
                    out=et[:],
                    in_=es_d.ap()[g * GROUP_E:(g + 1) * GROUP_E, :]
                        .rearrange("(p c) f -> p c f", p=P))
                pp = ps.tile([P, 512], F32, name="pp", tag="pp", bufs=2)
                for mm in range(4):
                    nc.tensor.matmul(
                        out=pp[32 * mm:32 * mm + 32, :],
                        lhsT=trib[:],
                        rhs=et[:, 8 * mm:8 * mm + 8, :]
                            .rearrange("p c f -> p (c f)"),
                        start=True, stop=True,
                        tile_position=(0, 32 * mm))
                if g == 2:
                    issue_node_loads()
                # ACT: PSUM -> bf16 table rows (duplicated 256B halves)
                for d in range(2):
                    nc.scalar.copy(
                        out=tabs[s][:, gg * 8:(gg + 1) * 8, d, :],
                        in_=pp[:].rearrange("p (c f) -> p c f", f=64))
                if gg != B_S[s] - 1:
                    continue
                # batch s table complete: issue SBUF->SBUF transposed gathers
                k0, KBB = KB0[s], KB_LIST[s]
                halves = [(0, KBB // 2), (KBB // 2, KBB)]
                for hh, (a, b) in enumerate(halves):
                    nidx = (b - a) * P
                    nc.gpsimd.dma_gather(
                        out_ap=phi[:, 16 + (k0 + a) * P: 16 + (k0 + b) * P]
                            .rearrange("p (o n) -> p o n", o=1),
                        in_ap=tabs[s][:].rearrange("p a b c -> p (a b c)"),
                        idxs_ap=hi_t[:, (k0 + a) * 8:(k0 + b) * 8],
                        num_idxs=nidx, num_idxs_reg=nidx, elem_size=128,
                        transpose=True, single_packet=GATHER_SINGLE_PACKET,
                        queue_num=(2 * s + hh) % 4 if GATHER_QUEUES == 4 else hh % GATHER_QUEUES,
                        sbuf_tokens_per_rank=128,
                        sbuf_free_dim_per_rank=256,
                        sbuf_free_dim_pad_per_rank=0,
                        sbuf_byte_offset=0)
                if s >= 1:
                    emit_meanA(s - 1)
                if s >= 2:
                    emit_batch_dense(s - 2)

            emit_meanA(S_BATCH - 1)
            for s in range(max(0, S_BATCH - 2), S_BATCH):
                emit_batch_dense(s)
            flush_pool(True)

            # ---- final: normalize + output projection
            den = sb.tile([GL, 4], F32, name="den")
            nc.vector.tensor_scalar_max(out=den[:], in0=pool_ps[:, 256:260],
                                        scalar1=1e-30)
            rden = sb.tile([GL, 4], F32, name="rden")
            nc.vector.reciprocal(out=rden[:], in_=den[:])
            pn = sb.tile([GL, 256], F32, name="pn")
            for hh in range(NUM_HEADS):
                nc.vector.tensor_scalar_mul(out=pn[:, 64 * hh:64 * hh + 64],
                                            in0=pool_ps[:, 64 * hh:64 * hh + 64],
                                            scalar1=rden[:, hh:hh + 1])
            pnT = sb.tile([P, 2, GL], F32, name="pnT")
            for i in range(2):
                ptp = ps.tile([P, GL], F32, name="ptp", tag="ptp", bufs=2)
                nc.tensor.transpose(out=ptp[:], in_=pn[:, i * P:(i + 1) * P],
                                    identity=idtf[:GL, :GL])
                nc.vector.tensor_copy(out=pnT[:, i, :], in_=ptp[:])
            ops_t = ps.tile([GL, 256], F32, name="ops_t", tag="ptp", bufs=2)
            for i in range(2):
                nc.tensor.matmul(out=ops_t[:], lhsT=pnT[:, i, :], rhs=owt[:, i, :],
                                 start=(i == 0), stop=(i == 1))
            osb = sb.tile([GL, 256], F32, name="osb")
            nc.vector.tensor_add(out=osb[:], in0=ops_t[:], in1=obt[:])
            nc.sync.dma_start(out=y_d.ap()[:, :], in_=osb[:])
            if DEBUG_DUMP:
                nc.sync.dma_start(out=phid_d.ap()[:, :], in_=phi[:])
                nc.sync.dma_start(
                    out=tabd_d.ap()[:, :],
                    in_=tabs[0][:].rearrange("p a b c -> p (a b c)"))
                nc.sync.dma_start(
                    out=augd_d.ap()[:, :],
                    in_=augT[:].rearrange("p a b -> p (a b)"))

    nc.finalize()
    return nc


_CACHE = {}


def _get_program(cfg):
    key = tuple(cfg["B_S"])
    if key not in _CACHE:
        _CACHE[key] = _build_program(cfg)
    return _CACHE[key]


def kernel(h, edge_index, edge_attr, batch, num_graphs,
           node_w, node_b, edge_w, edge_b, query, in_w, in_b, out_w, out_b,
           _trace=False, _numpy_only=False):
    import ml_dtypes
    h = np.asarray(h, np.float32)
    edge_attr = np.asarray(edge_attr, np.float32)
    batch_np = np.asarray(batch, np.int64)
    assert int(num_graphs) == G_TOTAL

    cores, cfg = _prep(h, edge_index, edge_attr, batch_np)
    Wbig = _fold_weights(np.asarray(node_w, np.float32), np.asarray(node_b, np.float32),
                         np.asarray(edge_w, np.float32), np.asarray(edge_b, np.float32),
                         np.asarray(query, np.float32), np.asarray(in_w, np.float32),
                         np.asarray(in_b, np.float32))
    out_w32 = np.asarray(out_w, np.float32)
    out_b32 = np.asarray(out_b, np.float32)
    if _numpy_only:
        return _numpy_device_model(cores, cfg, h, edge_attr, batch_np, Wbig,
                                   out_w32, out_b32)

    bf16 = ml_dtypes.bfloat16
    fp8 = ml_dtypes.float8_e4m3 if ES_FP8 else bf16
    tri = (np.arange(P)[:, None] // SLOT <= np.arange(SPC)[None, :]).astype(np.float32)
    idt = np.eye(P)
    iota = np.tile(np.arange(GL, dtype=np.float32)[None, :], (P, 1))
    shared = dict(
        tri=tri.astype(fp8),
        idtf=idt.astype(np.float32),
        iota=iota,
        wb1=Wbig[:128].astype(bf16), wb2=Wbig[128:].astype(bf16),
        owt=np.ascontiguousarray(out_w32.T),
        ob=np.tile(out_b32[None, :], (GL, 1)),
    )
    in_maps = []
    for c, st in enumerate(cores):
        NL = st["NL"]
        stream = np.zeros((cfg["E_PAD"], 64), np.float32)
        stream[st["epos"]] = edge_attr[st["esrc"]]
        hpad = np.zeros((NC_NODES, 128), np.float32)
        hpad[:NL] = h[st["n0"]:st["n1"]][st["perm"]]
        degp = np.zeros(NC_NODES, np.float32)
        degp[:NL] = st["deg"][st["perm"]]
        inv = (1.0 / np.maximum(degp, 1.0)).astype(np.float32)
        blp = np.full(NC_NODES, -1.0, np.float32)
        blp[:NL] = (batch_np[st["n0"]:st["n1"]] - GL * c)[st["perm"]]
        hase = (degp > 0).astype(np.float32)
        hone = np.stack([hase, np.ones(NC_NODES, np.float32)])
        h3 = np.ascontiguousarray(hpad.reshape(NKB, P, 128).transpose(2, 0, 1))
        in_maps.append(dict(
            es=stream.astype(fp8), h=h3.astype(bf16),
            bid=np.ascontiguousarray(blp.reshape(NKB, P).T),
            hi=_wrap_idx(st["hi_row"]),
            inv=np.tile(inv[None, :], (64, 1)).astype(bf16),
            mrep=np.tile(st["mrow"].astype(np.float32)[None, :], (64, 1)).astype(bf16),
            hone=hone.reshape(2, NKB, P).astype(bf16),
            **shared))

    from concourse.bass_utils import run_bass_kernel_spmd
    nc = _get_program(cfg)
    res = run_bass_kernel_spmd(nc, in_maps, core_ids=list(range(CORES)),
                               trace=_trace)
    out = np.concatenate([np.asarray(res.results[c]["y"], np.float32)
                          for c in range(CORES)])
    kernel._last_result = res
    return out.reshape(G_TOTAL, 1, 256)


# revision 31
# speedup vs baseline: 1.0091x; 1.0091x over previous
"""AttentionPooling GNN kernel for 8 Trainium2 NeuronCores.

Strategy (v2)
-------------
Graph-parallel sharding: 128 graphs -> 16 graphs per core; each core gets its
graphs' nodes and (re-grouped) edges.  Host does index preprocessing only
(edge permutation, packing, weight folding); all FLOPs on device.

Device algorithm (per core):
 1. Edge stream in fp8e4m3, sorted by destination node, padded to 4-edge
    slots inside 128-edge chunks (32 slots, no pad slot).  A shared
    triangular stationary (TriU4, fp8) turns chunk matmuls into slot-granular
    prefix sums P4 accumulated in PSUM fp32; 4 col-groups packed via
    tile_position.  ACT copies P4 to a per-batch SBUF table in bf16 with each
    (chunk,slot) row duplicated to 256B so it is gatherable.
 2. Nodes are processed in *packing order* (per-128-block, desc-degree).
    One SBUF->SBUF transposed dma_gather per batch-half pulls each node's
    run-end prefix row feature-major: phi[feat, node].  Because runs are
    back-to-back inside a chunk, the run-start prefix of node i is the
    run-end prefix of node i-1 (or exact zero at chunk starts), so
    A = phi_i - m_i * phi_{i-1} (m is a host-shipped 0/1 row-replicated
    mask) and meanA = A * inv_deg -- 3 DVE ops writing the wb2 stationary
    augT directly.  No lo-gather, no PE transposes.
 3. The linear chain (node proj + edge proj + v-proj + attention scores) is
    folded host-side into one [194, 260] matrix applied as two matmuls per
    128-node chunk: [h(128)] @ wb1 + [meanA(64)|has_edge|1] @ wb2
    -> [v(256) | scores(4)] in PSUM.
 4. w = exp(scores) (ACT); pr = [w*v | w] (DVE); pooling = matmul with the
    per-chunk one-hot graph-membership matrix accumulated in PSUM
    -> [16, 260] segment sums.
 5. pooled = U/denom; out = pooled @ out_w.T + out_b (fp32).
"""
import sys

sys.path.insert(0, "/opt/trn_rl_repo")

import numpy as np

NUM_HEADS = 4
G_TOTAL = 128
CORES = 8
GL = G_TOTAL // CORES       # graphs per core
P = 128                     # partitions
SLOT = 4                    # edges per slot
SPC = 32                    # slots per chunk (128 edges)
CPG = 32                    # chunks per group (4096 edges)
GROUP_E = P * CPG           # 4096 edges per group
KB_LIST = [8, 8, 8, 7, 7, 6, 5, 3]   # node chunks per gather batch
S_BATCH = len(KB_LIST)
KB0 = [0]
for _kb in KB_LIST:
    KB0.append(KB0[-1] + _kb)
NKB = KB0[-1]                        # 52 node chunks
NC_NODES = NKB * P                   # 6656 padded nodes per core
DENSE_LAG = 1
ES_FP8 = True
DEBUG_DUMP = False
GATHER_SINGLE_PACKET = True
GATHER_QUEUES = 4


# ----------------------------------------------------------------- host prep
def _pack_core(deg):
    """Pack node runs (padded to 4-edge slots) into 32-slot chunks in
    descending-degree order per 128-node block; no pad slots, runs
    back-to-back, no run straddles a chunk, each block starts a fresh chunk.

    Returns (perm [NL] packing order -> original local node,
             s0 [NL] global start slot (packing order, -1 for deg 0),
             m [NL] 1 if run does not start a chunk (packing order),
             blk_chunks [NB]).
    """
    NL = len(deg)
    r = (deg + SLOT - 1) // SLOT
    assert r.max(initial=0) <= SPC, "node degree > 128 unsupported"
    nblocks = (NL + P - 1) // P
    perm = np.zeros(NL, np.int64)
    s0 = np.full(NL, -1, np.int64)
    m = np.zeros(NL, np.int64)
    blk_chunks = np.zeros(nblocks, np.int64)
    cur = 0  # global slot cursor
    pos = 0
    for kb in range(nblocks):
        if cur % SPC:
            cur = (cur // SPC + 1) * SPC
        start_chunk = cur // SPC
        blk_nodes = list(range(kb * P, min((kb + 1) * P, NL)))
        blk_nodes.sort(key=lambda n: (-r[n], n))
        for n in blk_nodes:
            rn = int(r[n])
            perm[pos] = n
            if rn:
                if cur % SPC + rn > SPC:
                    cur = (cur // SPC + 1) * SPC
                s0[pos] = cur
                m[pos] = 1 if cur % SPC else 0
                cur += rn
            pos += 1
        blk_chunks[kb] = (cur + SPC - 1) // SPC - start_chunk
        cur = (cur + SPC - 1) // SPC * SPC
    return perm, s0, m, blk_chunks


def _prep(h, edge_index, edge_attr, batch):
    """Shard + pack. Returns per-core dict of host arrays + shared config."""
    row = np.asarray(edge_index[0], np.int64)
    batch = np.asarray(batch, np.int64)
    gstart = np.searchsorted(batch, np.arange(G_TOTAL + 1))
    order = np.argsort(row, kind="stable")
    row_s = row[order]

    cores = []
    for c in range(CORES):
        n0, n1 = int(gstart[GL * c]), int(gstart[GL * (c + 1)])
        NL = n1 - n0
        assert NL <= NC_NODES, (NL, NC_NODES)
        e0, e1 = np.searchsorted(row_s, [n0, n1])
        eord = order[e0:e1]
        lrow = row_s[e0:e1] - n0
        deg = np.bincount(lrow, minlength=NL)
        perm, s0, m, blk_chunks = _pack_core(deg)
        nb = len(blk_chunks)
        need = np.zeros(S_BATCH, np.int64)
        for s in range(S_BATCH):
            ch = blk_chunks[KB0[s]:min(KB0[s + 1], nb)].sum()
            need[s] = ch // CPG + 1          # >=1 spare chunk (zero row)
        cores.append(dict(n0=n0, n1=n1, NL=NL, eord=eord, lrow=lrow, deg=deg,
                          perm=perm, s0=s0, m=m, blk_chunks=blk_chunks,
                          need=need))
    B_S = [max(int(st["need"][s]) for st in cores) for s in range(S_BATCH)]
    G0 = [0]
    for b in B_S:
        G0.append(G0[-1] + b)
    NGROUPS = G0[-1]
    E_PAD = NGROUPS * GROUP_E
    assert max(B_S) * 1024 <= 32767

    for c, st in enumerate(cores):
        deg, perm, s0, m, blk_chunks = (st["deg"], st["perm"], st["s0"],
                                        st["m"], st["blk_chunks"])
        NL = st["NL"]
        nb = len(blk_chunks)
        # shift block-sequential slots to batch-aligned global slots
        kb_batch = np.searchsorted(np.asarray(KB0[1:]), np.arange(nb),
                                   side="right")
        blk_chunk0 = np.zeros(nb, np.int64)
        used = np.zeros(S_BATCH, np.int64)
        cur_chunk = 0
        for kb in range(nb):
            s = int(kb_batch[kb])
            if kb == KB0[s]:
                cur_chunk = G0[s] * CPG
            blk_chunk0[kb] = cur_chunk
            cur_chunk += blk_chunks[kb]
            used[s] = cur_chunk - G0[s] * CPG
            assert cur_chunk < G0[s + 1] * CPG  # strict: spare chunk exists
        orig_start = np.concatenate([[0], np.cumsum(blk_chunks)])[:-1]
        shift = (blk_chunk0 - orig_start) * SPC
        pos_blk = np.arange(NL) // P
        s0g = np.where(s0 >= 0, s0 + shift[pos_blk], -1)

        # edge stream positions: edges of node perm[i] occupy slots starting
        # at s0g[i]; swizzle to partition-major DRAM layout
        first_edge = np.concatenate([[0], np.cumsum(deg)])[:-1]
        degp = deg[perm]                       # degree in packing order
        has = degp > 0
        epos_base = np.repeat(SLOT * s0g[has], degp[has])
        within = np.concatenate(
            [np.arange(d) for d in degp[has]]) if has.any() else np.zeros(0, np.int64)
        src = np.repeat(first_edge[perm[has]], degp[has]) + within
        epos = epos_base + within
        assert epos.max(initial=-1) < E_PAD
        eg = epos // GROUP_E
        ep = epos % P
        ec = (epos % GROUP_E) // P
        st["epos"] = eg * GROUP_E + ep * CPG + ec
        st["esrc"] = st["eord"][src]

        # per-node (packing order) hi-row local table index + m flag
        def slot_to_idx(sl_g):
            chunk = sl_g // SPC
            sl = sl_g % SPC
            g = chunk // CPG
            s = np.searchsorted(np.asarray(G0[1:]), g, side="right")
            gg = g - np.asarray(G0)[s]
            ch = chunk % CPG
            return (gg * 8 + ch % 8) * P + 32 * (ch // 8) + sl

        hi = np.where(s0g >= 0, s0g + (degp + SLOT - 1) // SLOT - 1, 0)
        hi_idx = np.where(s0g >= 0, slot_to_idx(np.maximum(hi, 0)), 0)
        # zero-row (first unused chunk) per batch
        pos_batch = kb_batch[pos_blk]
        zero_idx = np.zeros(S_BATCH, np.int64)
        for s in range(S_BATCH):
            cu = int(used[s])
            zero_idx[s] = (cu // CPG * 8 + cu % CPG % 8) * P + 32 * (cu % CPG // 8)
        hi_idx = np.where(s0g >= 0, hi_idx, zero_idx[pos_batch])
        mm = np.where(s0g >= 0, m, 0)
        # pad nodes to NC_NODES (all use last batch's zero row, m=0)
        hi_row = np.concatenate([hi_idx, np.full(NC_NODES - NL,
                                                 zero_idx[S_BATCH - 1])])
        mrow = np.concatenate([mm, np.zeros(NC_NODES - NL, np.int64)])
        assert hi_row.max() < max(B_S) * 1024
        st.update(hi_row=hi_row.astype(np.int64), mrow=mrow)
    cfg = dict(B_S=B_S, G0=G0, NGROUPS=NGROUPS, E_PAD=E_PAD)
    return cores, cfg


def _wrap_idx(a, npart_rep=8):
    """[M] -> [128, M//16] int16, F-wrapped 16-row block replicated 8x."""
    mm = a.reshape(-1, 16).T.astype(np.int16)
    return np.tile(mm, (npart_rep, 1))


def _fold_weights(node_w, node_b, edge_w, edge_b, query, in_w, in_b):
    D = query.shape[-1]
    dh = D // NUM_HEADS
    wq, wk, wv = in_w[:D], in_w[D:2 * D], in_w[2 * D:]
    bq, bk, bv = in_b[:D], in_b[D:2 * D], in_b[2 * D:]
    q = (query[0] @ wq.T + bq).reshape(NUM_HEADS, dh)
    s_w = np.einsum("hj,hjd->dh", q, wk.reshape(NUM_HEADS, dh, D)) / np.sqrt(dh)
    s_b = np.einsum("hj,hj->h", q, bk.reshape(NUM_HEADS, dh)) / np.sqrt(dh)
    A1 = np.concatenate([node_w.T, edge_w.T, edge_b[None, :], node_b[None, :]],
                        axis=0)
    M2 = np.concatenate([wv.T, s_w], axis=1)
    Wbig = A1 @ M2                                     # [194, 260]
    Wbig[-1, :256] += bv
    Wbig[-1, 256:] += s_b
    return Wbig.astype(np.float32)


# ------------------------------------------------------- numpy device model
def _numpy_device_model(cores, cfg, h, edge_attr, batch, Wbig, out_w, out_b):
    """Bit-approximate emulation of the device program (fp8/bf16 rounding
    where the device rounds) — validates packing/indexing host-side."""
    import ml_dtypes
    bf = lambda x: x.astype(ml_dtypes.bfloat16).astype(np.float32)
    f8 = lambda x: x.astype(ml_dtypes.float8_e4m3).astype(np.float32)
    B_S, G0, E_PAD = cfg["B_S"], cfg["G0"], cfg["E_PAD"]
    outs = []
    for c, st in enumerate(cores):
        NL = st["NL"]
        stream = np.zeros((E_PAD, 64), np.float32)
        stream[st["epos"]] = edge_attr[st["esrc"]]
        streamq = f8(stream) if ES_FP8 else bf(stream)
        # per-batch P4 tables, bf16 rows
        tabs = []
        for s in range(S_BATCH):
            tab = np.zeros((B_S[s] * 1024, 64), np.float32)
            for gg in range(B_S[s]):
                g = G0[s] + gg
                for ch in range(CPG):
                    # chunk edges: global positions g*4096 + p*32 + ch for p
                    cdat = streamq[g * GROUP_E + np.arange(P) * CPG + ch]
                    pre = np.add.reduceat(cdat, np.arange(0, P, SLOT), 0).cumsum(0)
                    idx = (gg * 8 + ch % 8) * P + 32 * (ch // 8) + np.arange(SPC)
                    tab[idx] = pre
            tabs.append(bf(tab))
        # gather phi in packing order
        kb_of_pos = np.searchsorted(np.asarray(KB0[1:]), np.arange(NC_NODES) // P,
                                    side="right")
        phi = np.zeros((NC_NODES + 1, 64), np.float32)
        for i in range(NC_NODES):
            phi[i + 1] = tabs[kb_of_pos[i]][st["hi_row"][i]]
        mrep = st["mrow"].astype(np.float32)
        degp = np.zeros(NC_NODES, np.float32)
        degp[:NL] = st["deg"][st["perm"]]
        inv = 1.0 / np.maximum(degp, 1.0)
        t = bf(phi[:-1] * mrep[:, None])
        a = bf(phi[1:] - t)
        meanA = bf(a * inv[:, None])
        hase = (degp > 0).astype(np.float32)
        hpad = np.zeros((NC_NODES, 128), np.float32)
        hpad[:NL] = h[st["n0"]:st["n1"]][st["perm"]]
        xaug = np.concatenate([bf(hpad), meanA, bf(hase[:, None]),
                               np.ones((NC_NODES, 1), np.float32)], 1)
        vs = bf(xaug) @ bf(Wbig)
        v, sc = vs[:, :256], vs[:, 256:]
        w = np.exp(sc)
        bl = np.full(NC_NODES, -1, np.int64)
        bl[:NL] = (batch[st["n0"]:st["n1"]] - GL * c)[st["perm"]]
        onehot = (bl[:, None] == np.arange(GL)[None, :]).astype(np.float32)
        wv4 = np.concatenate(
            [bf(w[:, :, None] * v.reshape(-1, 4, 64)).reshape(-1, 256), bf(w)], 1)
        U = bf(onehot).T @ wv4
        den = np.maximum(U[:, 256:], 1e-30)
        pooled = U[:, :256].reshape(GL, 4, 64) / den[:, :, None]
        o = pooled.reshape(GL, 256) @ out_w.T + out_b
        outs.append(o)
    return np.concatenate(outs).reshape(G_TOTAL, 1, 256)


# ------------------------------------------------------------- bass program
def _build_program(cfg):
    import concourse.bacc as bacc
    import concourse.mybir as mybir
    import concourse.tile as tile

    F32 = mybir.dt.float32
    BF16 = mybir.dt.bfloat16
    FP8 = mybir.dt.float8e4 if ES_FP8 else mybir.dt.bfloat16
    I16 = mybir.dt.int16
    AF = mybir.ActivationFunctionType
    B_S, G0, NGROUPS, E_PAD = cfg["B_S"], cfg["G0"], cfg["NGROUPS"], cfg["E_PAD"]

    nc = bacc.Bacc("TRN2", num_devices=CORES, num_swdge_queues=4)
    es_d = nc.dram_tensor("es", [E_PAD, 64], FP8, kind="ExternalInput")
    h_d = nc.dram_tensor("h", [P, NKB, 128], BF16, kind="ExternalInput")
    bid_d = nc.dram_tensor("bid", [P, NKB], F32, kind="ExternalInput")
    hi_d = nc.dram_tensor("hi", [P, NC_NODES // 16], I16, kind="ExternalInput")
    inv_d = nc.dram_tensor("inv", [64, NC_NODES], BF16, kind="ExternalInput")
    mrep_d = nc.dram_tensor("mrep", [64, NC_NODES], BF16, kind="ExternalInput")
    hone_d = nc.dram_tensor("hone", [2, NKB, 128], BF16, kind="ExternalInput")
    tri_d = nc.dram_tensor("tri", [P, SPC], FP8, kind="ExternalInput")
    idtf_d = nc.dram_tensor("idtf", [P, P], F32, kind="ExternalInput")
    iota_d = nc.dram_tensor("iota", [P, GL], F32, kind="ExternalInput")
    wb1_d = nc.dram_tensor("wb1", [128, 260], BF16, kind="ExternalInput")
    wb2_d = nc.dram_tensor("wb2", [66, 260], BF16, kind="ExternalInput")
    owt_d = nc.dram_tensor("owt", [256, 256], F32, kind="ExternalInput")
    ob_d = nc.dram_tensor("ob", [GL, 256], F32, kind="ExternalInput")
    y_d = nc.dram_tensor("y", [GL, 256], F32, kind="ExternalOutput")
    if DEBUG_DUMP:
        phid_d = nc.dram_tensor("phid", [P, NC_NODES + 16], BF16,
                                kind="ExternalOutput")
        tabd_d = nc.dram_tensor("tabd", [P, B_S[0] * 1024], BF16,
                                kind="ExternalOutput")
        augd_d = nc.dram_tensor("augd", [66, NKB * 128], BF16,
                                kind="ExternalOutput")

    with tile.TileContext(nc) as tc:
        with tc.tile_pool(name="const", bufs=1) as cp, \
             tc.tile_pool(name="sb", bufs=3) as sb, \
             tc.tile_pool(name="big", bufs=1) as bigp, \
             tc.tile_pool(name="ps", bufs=2, space="PSUM") as ps, \
             tc.tile_pool(name="pacc", bufs=1, space="PSUM") as pacc:

            trib = cp.tile([P, SPC], FP8, name="trib")
            nc.sync.dma_start(out=trib[:], in_=tri_d.ap()[:, :])
            idtf = cp.tile([P, P], F32, name="idtf")
            nc.sync.dma_start(out=idtf[:], in_=idtf_d.ap()[:, :])
            iot = cp.tile([P, GL], F32, name="iot")
            nc.sync.dma_start(out=iot[:], in_=iota_d.ap()[:, :])
            wb1 = cp.tile([128, 260], BF16, name="wb1")
            nc.sync.dma_start(out=wb1[:], in_=wb1_d.ap()[:, :])
            wb2 = cp.tile([66, 260], BF16, name="wb2")
            nc.sync.dma_start(out=wb2[:], in_=wb2_d.ap()[:, :])
            owt = cp.tile([P, 2, 256], F32, name="owt")
            nc.sync.dma_start(out=owt[:],
                              in_=owt_d.ap()[:, :].rearrange("(i p) f -> p i f", p=P))
            obt = cp.tile([GL, 256], F32, name="obt")
            nc.sync.dma_start(out=obt[:], in_=ob_d.ap()[:, :])
            hi_t = cp.tile([P, NC_NODES // 16], I16, name="hi_t")
            nc.sync.dma_start(out=hi_t[:], in_=hi_d.ap()[:, :])

            # big node-side loads on SWDGE (gpsimd) queues: keeps the DMAHW
            # semaphore lanes free for the latency-critical es stream
            hsb = bigp.tile([P, NKB, 128], BF16, name="hsb")
            bid = bigp.tile([P, NKB], F32, name="bid")
            invr = bigp.tile([64, NC_NODES], BF16, name="invr")
            mrep = bigp.tile([64, NC_NODES], BF16, name="mrep")
            augT = bigp.tile([66, NKB, 128], BF16, name="augT")
            nc.gpsimd.dma_start(out=invr[:], in_=inv_d.ap()[:, :])
            nc.gpsimd.dma_start(out=mrep[:], in_=mrep_d.ap()[:, :])
            nc.gpsimd.dma_start(out=hsb[:], in_=h_d.ap()[:, :, :])
            nc.gpsimd.dma_start(out=bid[:], in_=bid_d.ap()[:, :])
            nc.gpsimd.dma_start(out=augT[64:66, :, :], in_=hone_d.ap()[:, :, :])
            memall = bigp.tile([P, NKB, GL], BF16, name="memall")
            # gather dst must stay 32B-aligned: zero column lives at 15,
            # gathered node i at column 16+i
            phi = bigp.tile([P, NC_NODES + 16], BF16, name="phi")
            nc.vector.memset(phi[:, 0:16], 0)
            tabs = [bigp.tile([P, B_S[s] * 8, 2, 64], BF16, name=f"tab{s}")
                    for s in range(S_BATCH)]

            pool_ps = pacc.tile([GL, 260], F32, name="pool_ps")

            def emit_meanA(s):
                k0, KBB = KB0[s], KB_LIST[s]
                n0c, n1c = k0 * P, (k0 + KBB) * P
                # meanA = (phi_i - m*phi_{i-1}) * inv  -> augT rows 0..63
                tmp = sb.tile([64, KBB * P], BF16, name="tmp", tag="tmp", bufs=2)
                nc.vector.tensor_tensor(out=tmp[:], in0=phi[0:64, 15 + n0c:15 + n1c],
                                        in1=mrep[:, n0c:n1c],
                                        op=mybir.AluOpType.mult)
                av = sb.tile([64, KBB * P], BF16, name="av", tag="av", bufs=2)
                nc.vector.tensor_tensor(out=av[:], in0=phi[0:64, 16 + n0c:16 + n1c],
                                        in1=tmp[:], op=mybir.AluOpType.subtract)
                nc.vector.tensor_tensor(
                    out=augT[0:64, k0:k0 + KBB, :].rearrange("p a b -> p (a b)"),
                    in0=av[:], in1=invr[:, n0c:n1c], op=mybir.AluOpType.mult)
                nc.vector.tensor_tensor(
                    out=memall[:, k0:k0 + KBB, :],
                    in0=iot[:].broadcast_to([P, GL, KBB]).rearrange("p g k -> p k g"),
                    in1=bid[:, k0:k0 + KBB].broadcast_to([P, KBB, GL]),
                    op=mybir.AluOpType.is_equal)

            pending_pool = []     # [(k, pr_tile)] single entry, lag-1

            def flush_pool(stop):
                if pending_pool:
                    k, pr = pending_pool.pop()
                    nc.tensor.matmul(out=pool_ps[:], lhsT=memall[:, k, :],
                                     rhs=pr[:], start=(k == 0), stop=stop)

            def emit_batch_dense(s):
                k0, KBB = KB0[s], KB_LIST[s]
                for k in range(k0, k0 + KBB):
                    vs = ps.tile([P, 260], F32, name="vs", tag="vs", bufs=3)
                    nc.tensor.matmul(out=vs[:], lhsT=hsb[:, k, :], rhs=wb1[:],
                                     start=True, stop=False)
                    nc.tensor.matmul(out=vs[:], lhsT=augT[:66, k, :], rhs=wb2[:],
                                     start=False, stop=True)
                    flush_pool(False)
                    wsb = sb.tile([P, 4], F32, name="wsb", tag="wsb", bufs=4)
                    nc.scalar.activation(out=wsb[:], in_=vs[:, 256:260], func=AF.Exp)
                    pr = sb.tile([P, 260], BF16, name="pr", tag="pr", bufs=4)
                    nc.vector.tensor_tensor(
                        out=pr[:, :256].rearrange("p (h f) -> p h f", h=NUM_HEADS),
                        in0=vs[:, :256].rearrange("p (h f) -> p h f", h=NUM_HEADS),
                        in1=wsb[:].broadcast_to([P, NUM_HEADS, 64]),
                        op=mybir.AluOpType.mult)
                    nc.vector.tensor_copy(out=pr[:, 256:260], in_=wsb[:])
                    pending_pool.append((k, pr))

            import bisect
            for g in range(NGROUPS):
                s = bisect.bisect_right(G0, g) - 1
                gg = g - G0[s]
                et = sb.tile([P, CPG, 64], FP8, name="et", tag="et", bufs=4)
                nc.sync.dma_start(
                    out=et[:],
                    in_=es_d.ap()[g * GROUP_E:(g + 1) * GROUP_E, :]
                        .rearrange("(p c) f -> p c f", p=P))
                pp = ps.tile([P, 512], F32, name="pp", tag="pp", bufs=2)
                for mm in range(4):
                    nc.tensor.matmul(
                        out=pp[32 * mm:32 * mm + 32, :],
                        lhsT=trib[:],
                        rhs=et[:, 8 * mm:8 * mm + 8, :]
                            .rearrange("p c f -> p (c f)"),
                        start=True, stop=True,
                        tile_position=(0, 32 * mm))
                # ACT: PSUM -> bf16 table rows (duplicated 256B halves)
                for d in range(2):
                    nc.scalar.copy(
                        out=tabs[s][:, gg * 8:(gg + 1) * 8, d, :],
                        in_=pp[:].rearrange("p (c f) -> p c f", f=64))
                if gg != B_S[s] - 1:
                    continue
                # batch s table complete: issue SBUF->SBUF transposed gathers
                k0, KBB = KB0[s], KB_LIST[s]
                halves = [(0, KBB // 2), (KBB // 2, KBB)]
                for hh, (a, b) in enumerate(halves):
                    nidx = (b - a) * P
                    nc.gpsimd.dma_gather(
                        out_ap=phi[:, 16 + (k0 + a) * P: 16 + (k0 + b) * P]
                            .rearrange("p (o n) -> p o n", o=1),
                        in_ap=tabs[s][:].rearrange("p a b c -> p (a b c)"),
                        idxs_ap=hi_t[:, (k0 + a) * 8:(k0 + b) * 8],
                        num_idxs=nidx, num_idxs_reg=nidx, elem_size=128,
                        transpose=True, single_packet=GATHER_SINGLE_PACKET,
                        queue_num=(2 * s + hh) % 4 if GATHER_QUEUES == 4 else hh % GATHER_QUEUES,
                        sbuf_tokens_per_rank=128,
                        sbuf_free_dim_per_rank=256,
                        sbuf_free_dim_pad_per_rank=0,
                        sbuf_byte_offset=0)
                if s >= 1:
                    emit_meanA(s - 1)
                if s >= 2:
                    emit_batch_dense(s - 2)

            emit_meanA(S_BATCH - 1)
            for s in range(max(0, S_BATCH - 2), S_BATCH):
                emit_batch_dense(s)
            flush_pool(True)

            # ---- final: normalize + output projection
            den = sb.tile([GL, 4], F32, name="den")
            nc.vector.tensor_scalar_max(out=den[:], in0=pool_ps[:, 256:260],
                                        scalar1=1e-30)
            rden = sb.tile([GL, 4], F32, name="rden")
            nc.vector.reciprocal(out=rden[:], in_=den[:])
            pn = sb.tile([GL, 256], F32, name="pn")
            for hh in range(NUM_HEADS):
                nc.vector.tensor_scalar_mul(out=pn[:, 64 * hh:64 * hh + 64],
                                            in0=pool_ps[:, 64 * hh:64 * hh + 64],
                                            scalar1=rden[:, hh:hh + 1])
            pnT = sb.tile([P, 2, GL], F32, name="pnT")
            for i in range(2):
                ptp = ps.tile([P, GL], F32, name="ptp", tag="ptp", bufs=2)
                nc.tensor.transpose(out=ptp[:], in_=pn[:, i * P:(i + 1) * P],
                                    identity=idtf[:GL, :GL])
                nc.vector.tensor_copy(out=pnT[:, i, :], in_=ptp[:])
            ops_t = ps.tile([GL, 256], F32, name="ops_t", tag="ptp", bufs=2)
            for i in range(2):
                nc.tensor.matmul(out=ops_t[:], lhsT=pnT[:, i, :], rhs=owt[:, i, :],
                                 start=(i == 0), stop=(i == 1))
            osb = sb.tile([GL, 256], F32, name="osb")
            nc.vector.tensor_add(out=osb[:], in0=ops_t[:], in1=obt[:])
            nc.sync.dma_start(out=y_d.ap()[:, :], in_=osb[:])
            if DEBUG_DUMP:
                nc.sync.dma_start(out=phid_d.ap()[:, :], in_=phi[:])
                nc.sync.dma_start(
                    out=tabd_d.ap()[:, :],
                    in_=tabs[0][:].rearrange("p a b c -> p (a b c)"))
                nc.sync.dma_start(
                    out=augd_d.ap()[:, :],
                    in_=augT[:].rearrange("p a b -> p (a b)"))

    nc.finalize()
    return nc


_CACHE = {}


def _get_program(cfg):
    key = tuple(cfg["B_S"])
    if key not in _CACHE:
        _CACHE[key] = _build_program(cfg)
    return _CACHE[key]


def kernel(h, edge_index, edge_attr, batch, num_graphs,
           node_w, node_b, edge_w, edge_b, query, in_w, in_b, out_w, out_b,
           _trace=False, _numpy_only=False):
    import ml_dtypes
    h = np.asarray(h, np.float32)
    edge_attr = np.asarray(edge_attr, np.float32)
    batch_np = np.asarray(batch, np.int64)
    assert int(num_graphs) == G_TOTAL

    cores, cfg = _prep(h, edge_index, edge_attr, batch_np)
    Wbig = _fold_weights(np.asarray(node_w, np.float32), np.asarray(node_b, np.float32),
                         np.asarray(edge_w, np.float32), np.asarray(edge_b, np.float32),
                         np.asarray(query, np.float32), np.asarray(in_w, np.float32),
                         np.asarray(in_b, np.float32))
    out_w32 = np.asarray(out_w, np.float32)
    out_b32 = np.asarray(out_b, np.float32)
    if _numpy_only:
        return _numpy_device_model(cores, cfg, h, edge_attr, batch_np, Wbig,
                                   out_w32, out_b32)

    bf16 = ml_dtypes.bfloat16
    fp8 = ml_dtypes.float8_e4m3 if ES_FP8 else bf16
    tri = (np.arange(P)[:, None] // SLOT <= np.arange(SPC)[None, :]).astype(np.float32)
    idt = np.eye(P)
    iota = np.tile(np.arange(GL, dtype=np.float32)[None, :], (P, 1))
    shared = dict(
        tri=tri.astype(fp8),
        idtf=idt.astype(np.float32),
        iota=iota,
        wb1=Wbig[:128].astype(bf16), wb2=Wbig[128:].astype(bf16),
        owt=np.ascontiguousarray(out_w32.T),
        ob=np.tile(out_b32[None, :], (GL, 1)),
    )
    in_maps = []
    for c, st in enumerate(cores):
        NL = st["NL"]
        stream = np.zeros((cfg["E_PAD"], 64), np.float32)
        stream[st["epos"]] = edge_attr[st["esrc"]]
        hpad = np.zeros((NC_NODES, 128), np.float32)
        hpad[:NL] = h[st["n0"]:st["n1"]][st["perm"]]
        degp = np.zeros(NC_NODES, np.float32)
        degp[:NL] = st["deg"][st["perm"]]
        inv = (1.0 / np.maximum(degp, 1.0)).astype(np.float32)
        blp = np.full(NC_NODES, -1.0, np.float32)
        blp[:NL] = (batch_np[st["n0"]:st["n1"]] - GL * c)[st["perm"]]
        hase = (degp > 0).astype(np.float32)
        hone = np.stack([hase, np.ones(NC_NODES, np.float32)])
        h3 = np.ascontiguousarray(hpad.reshape(NKB, P, 128).transpose(2, 0, 1))
        in_maps.append(dict(
            es=stream.astype(fp8), h=h3.astype(bf16),
            bid=np.ascontiguousarray(blp.reshape(NKB, P).T),
            hi=_wrap_idx(st["hi_row"]),
            inv=np.tile(inv[None, :], (64, 1)).astype(bf16),
            mrep=np.tile(st["mrow"].astype(np.float32)[None, :], (64, 1)).astype(bf16),
            hone=hone.reshape(2, NKB, P).astype(bf16),
            **shared))

    from concourse.bass_utils import run_bass_kernel_spmd
    nc = _get_program(cfg)
    res = run_bass_kernel_spmd(nc, in_maps, core_ids=list(range(CORES)),
                               trace=_trace)
    out = np.concatenate([np.asarray(res.results[c]["y"], np.float32)
                          for c in range(CORES)])
    kernel._last_result = res
    return out.reshape(G_TOTAL, 1, 256)


# revision 37
# speedup vs baseline: 1.1678x; 1.1573x over previous
"""AttentionPooling GNN kernel for 8 Trainium2 NeuronCores.

Strategy (v2)
-------------
Graph-parallel sharding: 128 graphs -> 16 graphs per core; each core gets its
graphs' nodes and (re-grouped) edges.  Host does index preprocessing only
(edge permutation, packing, weight folding); all FLOPs on device.

Device algorithm (per core):
 1. Edge stream in fp8e4m3, sorted by destination node, padded to 4-edge
    slots inside 128-edge chunks (32 slots, no pad slot).  A shared
    triangular stationary (TriU4, fp8) turns chunk matmuls into slot-granular
    prefix sums P4 accumulated in PSUM fp32; 4 col-groups packed via
    tile_position.  ACT copies P4 to a per-batch SBUF table in bf16 with each
    (chunk,slot) row duplicated to 256B so it is gatherable.
 2. Nodes are processed in *packing order* (per-128-block, desc-degree).
    One SBUF->SBUF transposed dma_gather per batch-half pulls each node's
    run-end prefix row feature-major: phi[feat, node].  Because runs are
    back-to-back inside a chunk, the run-start prefix of node i is the
    run-end prefix of node i-1 (or exact zero at chunk starts), so
    A = phi_i - m_i * phi_{i-1} (m is a host-shipped 0/1 row-replicated
    mask) and meanA = A * inv_deg -- 3 DVE ops writing the wb2 stationary
    augT directly.  No lo-gather, no PE transposes.
 3. The linear chain (node proj + edge proj + v-proj + attention scores) is
    folded host-side into one [194, 260] matrix applied as two matmuls per
    128-node chunk: [h(128)] @ wb1 + [meanA(64)|has_edge|1] @ wb2
    -> [v(256) | scores(4)] in PSUM.
 4. w = exp(scores) (ACT); pr = [w*v | w] (DVE); pooling = matmul with the
    per-chunk one-hot graph-membership matrix accumulated in PSUM
    -> [16, 260] segment sums.
 5. pooled = U/denom; out = pooled @ out_w.T + out_b (fp32).
"""
import sys

sys.path.insert(0, "/opt/trn_rl_repo")

import numpy as np

NUM_HEADS = 4
G_TOTAL = 128
CORES = 8
GL = G_TOTAL // CORES       # graphs per core
P = 128                     # partitions
SLOT = 4                    # edges per slot
SPC = 32                    # slots per chunk (128 edges)
CPG = 32                    # chunks per group (4096 edges)
GROUP_E = P * CPG           # 4096 edges per group
KB_LIST = [8, 8, 8, 7, 7, 6, 5, 3]   # node chunks per gather batch
S_BATCH = len(KB_LIST)
KB0 = [0]
for _kb in KB_LIST:
    KB0.append(KB0[-1] + _kb)
NKB = KB0[-1]                        # 52 node chunks
NC_NODES = NKB * P                   # 6656 padded nodes per core
DENSE_LAG = 1
ES_FP8 = True
DEBUG_DUMP = False
GATHER_SINGLE_PACKET = True
GATHER_QUEUES = 4


# ----------------------------------------------------------------- host prep
def _pack_core(deg):
    """Pack node runs (padded to 4-edge slots) into 32-slot chunks in
    descending-degree order per 128-node block; no pad slots, runs
    back-to-back, no run straddles a chunk, each block starts a fresh chunk.

    Returns (perm [NL] packing order -> original local node,
             s0 [NL] global start slot (packing order, -1 for deg 0),
             m [NL] 1 if run does not start a chunk (packing order),
             blk_chunks [NB]).
    """
    NL = len(deg)
    r = (deg + SLOT - 1) // SLOT
    assert r.max(initial=0) <= SPC, "node degree > 128 unsupported"
    nblocks = (NL + P - 1) // P
    perm = np.zeros(NL, np.int64)
    s0 = np.full(NL, -1, np.int64)
    m = np.zeros(NL, np.int64)
    blk_chunks = np.zeros(nblocks, np.int64)
    cur = 0  # global slot cursor
    pos = 0
    for kb in range(nblocks):
        if cur % SPC:
            cur = (cur // SPC + 1) * SPC
        start_chunk = cur // SPC
        blk_nodes = list(range(kb * P, min((kb + 1) * P, NL)))
        blk_nodes.sort(key=lambda n: (-r[n], n))
        for n in blk_nodes:
            rn = int(r[n])
            perm[pos] = n
            if rn:
                if cur % SPC + rn > SPC:
                    cur = (cur // SPC + 1) * SPC
                s0[pos] = cur
                m[pos] = 1 if cur % SPC else 0
                cur += rn
            pos += 1
        blk_chunks[kb] = (cur + SPC - 1) // SPC - start_chunk
        cur = (cur + SPC - 1) // SPC * SPC
    return perm, s0, m, blk_chunks


def _prep(h, edge_index, edge_attr, batch):
    """Shard + pack. Returns per-core dict of host arrays + shared config."""
    row = np.asarray(edge_index[0], np.int64)
    batch = np.asarray(batch, np.int64)
    gstart = np.searchsorted(batch, np.arange(G_TOTAL + 1))
    order = np.argsort(row, kind="stable")
    row_s = row[order]

    cores = []
    for c in range(CORES):
        n0, n1 = int(gstart[GL * c]), int(gstart[GL * (c + 1)])
        NL = n1 - n0
        assert NL <= NC_NODES, (NL, NC_NODES)
        e0, e1 = np.searchsorted(row_s, [n0, n1])
        eord = order[e0:e1]
        lrow = row_s[e0:e1] - n0
        deg = np.bincount(lrow, minlength=NL)
        perm, s0, m, blk_chunks = _pack_core(deg)
        nb = len(blk_chunks)
        need = np.zeros(S_BATCH, np.int64)
        for s in range(S_BATCH):
            ch = blk_chunks[KB0[s]:min(KB0[s + 1], nb)].sum()
            need[s] = ch // CPG + 1          # >=1 spare chunk (zero row)
        cores.append(dict(n0=n0, n1=n1, NL=NL, eord=eord, lrow=lrow, deg=deg,
                          perm=perm, s0=s0, m=m, blk_chunks=blk_chunks,
                          need=need))
    B_S = [max(int(st["need"][s]) for st in cores) for s in range(S_BATCH)]
    G0 = [0]
    for b in B_S:
        G0.append(G0[-1] + b)
    NGROUPS = G0[-1]
    E_PAD = NGROUPS * GROUP_E
    assert max(B_S) * 1024 <= 32767

    for c, st in enumerate(cores):
        deg, perm, s0, m, blk_chunks = (st["deg"], st["perm"], st["s0"],
                                        st["m"], st["blk_chunks"])
        NL = st["NL"]
        nb = len(blk_chunks)
        # shift block-sequential slots to batch-aligned global slots
        kb_batch = np.searchsorted(np.asarray(KB0[1:]), np.arange(nb),
                                   side="right")
        blk_chunk0 = np.zeros(nb, np.int64)
        used = np.zeros(S_BATCH, np.int64)
        cur_chunk = 0
        for kb in range(nb):
            s = int(kb_batch[kb])
            if kb == KB0[s]:
                cur_chunk = G0[s] * CPG
            blk_chunk0[kb] = cur_chunk
            cur_chunk += blk_chunks[kb]
            used[s] = cur_chunk - G0[s] * CPG
            assert cur_chunk < G0[s + 1] * CPG  # strict: spare chunk exists
        orig_start = np.concatenate([[0], np.cumsum(blk_chunks)])[:-1]
        shift = (blk_chunk0 - orig_start) * SPC
        pos_blk = np.arange(NL) // P
        s0g = np.where(s0 >= 0, s0 + shift[pos_blk], -1)

        # edge stream positions: edges of node perm[i] occupy slots starting
        # at s0g[i]; swizzle to partition-major DRAM layout
        first_edge = np.concatenate([[0], np.cumsum(deg)])[:-1]
        degp = deg[perm]                       # degree in packing order
        has = degp > 0
        epos_base = np.repeat(SLOT * s0g[has], degp[has])
        within = np.concatenate(
            [np.arange(d) for d in degp[has]]) if has.any() else np.zeros(0, np.int64)
        src = np.repeat(first_edge[perm[has]], degp[has]) + within
        epos = epos_base + within
        assert epos.max(initial=-1) < E_PAD
        eg = epos // GROUP_E
        ep = epos % P
        ec = (epos % GROUP_E) // P
        st["epos"] = eg * GROUP_E + ep * CPG + ec
        st["esrc"] = st["eord"][src]

        # per-node (packing order) hi-row local table index + m flag
        def slot_to_idx(sl_g):
            chunk = sl_g // SPC
            sl = sl_g % SPC
            g = chunk // CPG
            s = np.searchsorted(np.asarray(G0[1:]), g, side="right")
            gg = g - np.asarray(G0)[s]
            ch = chunk % CPG
            return (gg * 8 + ch % 8) * P + 32 * (ch // 8) + sl

        hi = np.where(s0g >= 0, s0g + (degp + SLOT - 1) // SLOT - 1, 0)
        hi_idx = np.where(s0g >= 0, slot_to_idx(np.maximum(hi, 0)), 0)
        # zero-row (first unused chunk) per batch
        pos_batch = kb_batch[pos_blk]
        zero_idx = np.zeros(S_BATCH, np.int64)
        for s in range(S_BATCH):
            cu = int(used[s])
            zero_idx[s] = (cu // CPG * 8 + cu % CPG % 8) * P + 32 * (cu % CPG // 8)
        hi_idx = np.where(s0g >= 0, hi_idx, zero_idx[pos_batch])
        mm = np.where(s0g >= 0, m, 0)
        # pad nodes to NC_NODES (all use last batch's zero row, m=0)
        hi_row = np.concatenate([hi_idx, np.full(NC_NODES - NL,
                                                 zero_idx[S_BATCH - 1])])
        mrow = np.concatenate([mm, np.zeros(NC_NODES - NL, np.int64)])
        assert hi_row.max() < max(B_S) * 1024
        st.update(hi_row=hi_row.astype(np.int64), mrow=mrow)
    cfg = dict(B_S=B_S, G0=G0, NGROUPS=NGROUPS, E_PAD=E_PAD)
    return cores, cfg


def _wrap_idx(a, npart_rep=8):
    """[M] -> [128, M//16] int16, F-wrapped 16-row block replicated 8x."""
    mm = a.reshape(-1, 16).T.astype(np.int16)
    return np.tile(mm, (npart_rep, 1))


def _fold_weights(node_w, node_b, edge_w, edge_b, query, in_w, in_b):
    D = query.shape[-1]
    dh = D // NUM_HEADS
    wq, wk, wv = in_w[:D], in_w[D:2 * D], in_w[2 * D:]
    bq, bk, bv = in_b[:D], in_b[D:2 * D], in_b[2 * D:]
    q = (query[0] @ wq.T + bq).reshape(NUM_HEADS, dh)
    s_w = np.einsum("hj,hjd->dh", q, wk.reshape(NUM_HEADS, dh, D)) / np.sqrt(dh)
    s_b = np.einsum("hj,hj->h", q, bk.reshape(NUM_HEADS, dh)) / np.sqrt(dh)
    A1 = np.concatenate([node_w.T, edge_w.T, edge_b[None, :], node_b[None, :]],
                        axis=0)
    M2 = np.concatenate([wv.T, s_w], axis=1)
    Wbig = A1 @ M2                                     # [194, 260]
    Wbig[-1, :256] += bv
    Wbig[-1, 256:] += s_b
    return Wbig.astype(np.float32)


# ------------------------------------------------------- numpy device model
def _numpy_device_model(cores, cfg, h, edge_attr, batch, Wbig, out_w, out_b):
    """Bit-approximate emulation of the device program (fp8/bf16 rounding
    where the device rounds) — validates packing/indexing host-side."""
    import ml_dtypes
    bf = lambda x: x.astype(ml_dtypes.bfloat16).astype(np.float32)
    f8 = lambda x: x.astype(ml_dtypes.float8_e4m3).astype(np.float32)
    B_S, G0, E_PAD = cfg["B_S"], cfg["G0"], cfg["E_PAD"]
    outs = []
    for c, st in enumerate(cores):
        NL = st["NL"]
        stream = np.zeros((E_PAD, 64), np.float32)
        stream[st["epos"]] = edge_attr[st["esrc"]]
        streamq = f8(stream) if ES_FP8 else bf(stream)
        # per-batch P4 tables, bf16 rows
        tabs = []
        for s in range(S_BATCH):
            tab = np.zeros((B_S[s] * 1024, 64), np.float32)
            for gg in range(B_S[s]):
                g = G0[s] + gg
                for ch in range(CPG):
                    # chunk edges: global positions g*4096 + p*32 + ch for p
                    cdat = streamq[g * GROUP_E + np.arange(P) * CPG + ch]
                    pre = np.add.reduceat(cdat, np.arange(0, P, SLOT), 0).cumsum(0)
                    idx = (gg * 8 + ch % 8) * P + 32 * (ch // 8) + np.arange(SPC)
                    tab[idx] = pre
            tabs.append(bf(tab))
        # gather phi in packing order
        kb_of_pos = np.searchsorted(np.asarray(KB0[1:]), np.arange(NC_NODES) // P,
                                    side="right")
        phi = np.zeros((NC_NODES + 1, 64), np.float32)
        for i in range(NC_NODES):
            phi[i + 1] = tabs[kb_of_pos[i]][st["hi_row"][i]]
        mrep = st["mrow"].astype(np.float32)
        degp = np.zeros(NC_NODES, np.float32)
        degp[:NL] = st["deg"][st["perm"]]
        inv = 1.0 / np.maximum(degp, 1.0)
        t = bf(phi[:-1] * mrep[:, None])
        a = bf(phi[1:] - t)
        meanA = bf(a * inv[:, None])
        hase = (degp > 0).astype(np.float32)
        hpad = np.zeros((NC_NODES, 128), np.float32)
        hpad[:NL] = h[st["n0"]:st["n1"]][st["perm"]]
        xaug = np.concatenate([bf(hpad), meanA, bf(hase[:, None]),
                               np.ones((NC_NODES, 1), np.float32)], 1)
        vs = bf(xaug) @ bf(Wbig)
        v, sc = vs[:, :256], vs[:, 256:]
        w = np.exp(sc)
        bl = np.full(NC_NODES, -1, np.int64)
        bl[:NL] = (batch[st["n0"]:st["n1"]] - GL * c)[st["perm"]]
        onehot = (bl[:, None] == np.arange(GL)[None, :]).astype(np.float32)
        wv4 = np.concatenate(
            [bf(w[:, :, None] * v.reshape(-1, 4, 64)).reshape(-1, 256), bf(w)], 1)
        U = bf(onehot).T @ wv4
        den = np.maximum(U[:, 256:], 1e-30)
        pooled = U[:, :256].reshape(GL, 4, 64) / den[:, :, None]
        o = pooled.reshape(GL, 256) @ out_w.T + out_b
        outs.append(o)
    return np.concatenate(outs).reshape(G_TOTAL, 1, 256)


# ------------------------------------------------------------- bass program
def _build_program(cfg):
    import concourse.bacc as bacc
    import concourse.mybir as mybir
    import concourse.tile as tile

    F32 = mybir.dt.float32
    BF16 = mybir.dt.bfloat16
    FP8 = mybir.dt.float8e4 if ES_FP8 else mybir.dt.bfloat16
    I16 = mybir.dt.int16
    AF = mybir.ActivationFunctionType
    B_S, G0, NGROUPS, E_PAD = cfg["B_S"], cfg["G0"], cfg["NGROUPS"], cfg["E_PAD"]

    nc = bacc.Bacc("TRN2", num_devices=CORES, num_swdge_queues=4)
    es_d = nc.dram_tensor("es", [E_PAD, 64], FP8, kind="ExternalInput")
    h_d = nc.dram_tensor("h", [P, NKB, 128], BF16, kind="ExternalInput")
    bid_d = nc.dram_tensor("bid", [P, NKB], F32, kind="ExternalInput")
    hi_d = nc.dram_tensor("hi", [P, NC_NODES // 16], I16, kind="ExternalInput")
    inv_d = nc.dram_tensor("inv", [64, NC_NODES], BF16, kind="ExternalInput")
    mrep_d = nc.dram_tensor("mrep", [64, NC_NODES], BF16, kind="ExternalInput")
    hone_d = nc.dram_tensor("hone", [2, NKB, 128], BF16, kind="ExternalInput")
    tri_d = nc.dram_tensor("tri", [P, SPC], FP8, kind="ExternalInput")
    idtf_d = nc.dram_tensor("idtf", [P, P], F32, kind="ExternalInput")
    iota_d = nc.dram_tensor("iota", [P, GL], F32, kind="ExternalInput")
    wb1_d = nc.dram_tensor("wb1", [128, 260], BF16, kind="ExternalInput")
    wb2_d = nc.dram_tensor("wb2", [66, 260], BF16, kind="ExternalInput")
    owt_d = nc.dram_tensor("owt", [256, 256], F32, kind="ExternalInput")
    ob_d = nc.dram_tensor("ob", [GL, 256], F32, kind="ExternalInput")
    y_d = nc.dram_tensor("y", [GL, 256], F32, kind="ExternalOutput")
    if DEBUG_DUMP:
        phid_d = nc.dram_tensor("phid", [P, NC_NODES + 16], BF16,
                                kind="ExternalOutput")
        tabd_d = nc.dram_tensor("tabd", [P, B_S[0] * 1024], BF16,
                                kind="ExternalOutput")
        augd_d = nc.dram_tensor("augd", [66, NKB * 128], BF16,
                                kind="ExternalOutput")

    with tile.TileContext(nc) as tc:
        with tc.tile_pool(name="const", bufs=1) as cp, \
             tc.tile_pool(name="sb", bufs=3) as sb, \
             tc.tile_pool(name="big", bufs=1) as bigp, \
             tc.tile_pool(name="ps", bufs=2, space="PSUM") as ps, \
             tc.tile_pool(name="pacc", bufs=1, space="PSUM") as pacc:

            # es load plan: pair groups (batch-aligned) to halve SP dispatch
            # serialization; the first two loads are hoisted before all other
            # consts so the PE starts within ~10us
            loads = []
            for s in range(S_BATCH):
                g = G0[s]
                while g < G0[s + 1]:
                    n = min(2, G0[s + 1] - g)
                    loads.append((g, n))
                    g += n
            et_tiles = {}

            def load_et(gi):
                g, n = loads[gi]
                et = sb.tile([P, 2, CPG, 64], FP8, name="et", tag="et", bufs=4)
                nc.sync.dma_start(
                    out=et[:, :n, :, :],
                    in_=es_d.ap()[g * GROUP_E:(g + n) * GROUP_E, :]
                        .rearrange("(n p c) f -> p n c f", p=P, n=n))
                for k in range(n):
                    et_tiles[g + k] = (et, k)

            trib = cp.tile([P, SPC], FP8, name="trib")
            nc.sync.dma_start(out=trib[:], in_=tri_d.ap()[:, :])
            load_et(0)
            load_et(1)
            wb1 = cp.tile([128, 260], BF16, name="wb1")
            nc.sync.dma_start(out=wb1[:], in_=wb1_d.ap()[:, :])
            wb2 = cp.tile([66, 260], BF16, name="wb2")
            nc.sync.dma_start(out=wb2[:], in_=wb2_d.ap()[:, :])
            iot = cp.tile([P, GL], F32, name="iot")
            nc.sync.dma_start(out=iot[:], in_=iota_d.ap()[:, :])
            hi_t = cp.tile([P, NC_NODES // 16], I16, name="hi_t")
            nc.sync.dma_start(out=hi_t[:], in_=hi_d.ap()[:, :])

            # big node-side loads on the ACT HWDGE queue: dispatch lands after
            # the hoisted et loads are in flight
            hsb = bigp.tile([P, NKB, 128], BF16, name="hsb")
            bid = bigp.tile([P, NKB], F32, name="bid")
            invr = bigp.tile([64, NC_NODES], BF16, name="invr")
            mrep = bigp.tile([64, NC_NODES], BF16, name="mrep")
            augT = bigp.tile([66, NKB, 128], BF16, name="augT")
            nc.scalar.dma_start(out=invr[:], in_=inv_d.ap()[:, :])
            nc.scalar.dma_start(out=mrep[:], in_=mrep_d.ap()[:, :])
            nc.scalar.dma_start(out=hsb[:], in_=h_d.ap()[:, :, :])
            nc.scalar.dma_start(out=bid[:], in_=bid_d.ap()[:, :])
            nc.scalar.dma_start(out=augT[64:66, :, :], in_=hone_d.ap()[:, :, :])
            memall = bigp.tile([P, NKB, GL], BF16, name="memall")
            # gather dst must stay 32B-aligned: zero column lives at 15,
            # gathered node i at column 16+i
            phi = bigp.tile([P, NC_NODES + 16], BF16, name="phi")
            nc.vector.memset(phi[:, 0:16], 0)
            tabs = [bigp.tile([P, B_S[s] * 8, 2, 64], BF16, name=f"tab{s}")
                    for s in range(S_BATCH)]

            pool_ps = pacc.tile([GL, 260], F32, name="pool_ps")

            def emit_meanA(s):
                k0, KBB = KB0[s], KB_LIST[s]
                n0c, n1c = k0 * P, (k0 + KBB) * P
                # meanA = (phi_i - m*phi_{i-1}) * inv  -> augT rows 0..63
                tmp = sb.tile([64, KBB * P], BF16, name="tmp", tag="tmp", bufs=2)
                nc.vector.tensor_tensor(out=tmp[:], in0=phi[0:64, 15 + n0c:15 + n1c],
                                        in1=mrep[:, n0c:n1c],
                                        op=mybir.AluOpType.mult)
                av = sb.tile([64, KBB * P], BF16, name="av", tag="av", bufs=2)
                nc.vector.tensor_tensor(out=av[:], in0=phi[0:64, 16 + n0c:16 + n1c],
                                        in1=tmp[:], op=mybir.AluOpType.subtract)
                nc.vector.tensor_tensor(
                    out=augT[0:64, k0:k0 + KBB, :].rearrange("p a b -> p (a b)"),
                    in0=av[:], in1=invr[:, n0c:n1c], op=mybir.AluOpType.mult)
                nc.vector.tensor_tensor(
                    out=memall[:, k0:k0 + KBB, :],
                    in0=iot[:].broadcast_to([P, GL, KBB]).rearrange("p g k -> p k g"),
                    in1=bid[:, k0:k0 + KBB].broadcast_to([P, KBB, GL]),
                    op=mybir.AluOpType.is_equal)

            pending_pool = []     # [(k, pr_tile)] single entry, lag-1

            def flush_pool(stop):
                if pending_pool:
                    k, pr = pending_pool.pop()
                    nc.tensor.matmul(out=pool_ps[:], lhsT=memall[:, k, :],
                                     rhs=pr[:], start=(k == 0), stop=stop)

            def emit_batch_dense(s):
                k0, KBB = KB0[s], KB_LIST[s]
                for k in range(k0, k0 + KBB):
                    vs = ps.tile([P, 260], F32, name="vs", tag="vs", bufs=3)
                    nc.tensor.matmul(out=vs[:], lhsT=hsb[:, k, :], rhs=wb1[:],
                                     start=True, stop=False)
                    nc.tensor.matmul(out=vs[:], lhsT=augT[:66, k, :], rhs=wb2[:],
                                     start=False, stop=True)
                    flush_pool(False)
                    wsb = sb.tile([P, 4], F32, name="wsb", tag="wsb", bufs=4)
                    nc.scalar.activation(out=wsb[:], in_=vs[:, 256:260], func=AF.Exp)
                    pr = sb.tile([P, 260], BF16, name="pr", tag="pr", bufs=4)
                    nc.vector.tensor_tensor(
                        out=pr[:, :256].rearrange("p (h f) -> p h f", h=NUM_HEADS),
                        in0=vs[:, :256].rearrange("p (h f) -> p h f", h=NUM_HEADS),
                        in1=wsb[:].broadcast_to([P, NUM_HEADS, 64]),
                        op=mybir.AluOpType.mult)
                    nc.vector.tensor_copy(out=pr[:, 256:260], in_=wsb[:])
                    pending_pool.append((k, pr))

            import bisect
            g_to_load = {}
            for gi, (g0_, n_) in enumerate(loads):
                for k in range(n_):
                    g_to_load[g0_ + k] = gi
            next_load = 2
            for g in range(NGROUPS):
                s = bisect.bisect_right(G0, g) - 1
                gg = g - G0[s]
                if g_to_load[g] >= next_load:
                    raise AssertionError("load not issued")
                et3, koff = et_tiles[g]
                if next_load < len(loads) and g == loads[next_load - 2][0]:
                    load_et(next_load)
                    next_load += 1
                pp = ps.tile([P, 512], F32, name="pp", tag="pp", bufs=2)
                for mm in range(4):
                    nc.tensor.matmul(
                        out=pp[32 * mm:32 * mm + 32, :],
                        lhsT=trib[:],
                        rhs=et3[:, koff, 8 * mm:8 * mm + 8, :]
                            .rearrange("p c f -> p (c f)"),
                        start=True, stop=True,
                        tile_position=(0, 32 * mm))
                # ACT: PSUM -> bf16 table rows (duplicated 256B halves)
                for d in range(2):
                    nc.scalar.copy(
                        out=tabs[s][:, gg * 8:(gg + 1) * 8, d, :],
                        in_=pp[:].rearrange("p (c f) -> p c f", f=64))
                if gg != B_S[s] - 1:
                    continue
                # batch s table complete: issue SBUF->SBUF transposed gathers
                k0, KBB = KB0[s], KB_LIST[s]
                halves = [(0, KBB // 2), (KBB // 2, KBB)]
                for hh, (a, b) in enumerate(halves):
                    nidx = (b - a) * P
                    nc.gpsimd.dma_gather(
                        out_ap=phi[:, 16 + (k0 + a) * P: 16 + (k0 + b) * P]
                            .rearrange("p (o n) -> p o n", o=1),
                        in_ap=tabs[s][:].rearrange("p a b c -> p (a b c)"),
                        idxs_ap=hi_t[:, (k0 + a) * 8:(k0 + b) * 8],
                        num_idxs=nidx, num_idxs_reg=nidx, elem_size=128,
                        transpose=True, single_packet=GATHER_SINGLE_PACKET,
                        queue_num=(2 * s + hh) % 4 if GATHER_QUEUES == 4 else hh % GATHER_QUEUES,
                        sbuf_tokens_per_rank=128,
                        sbuf_free_dim_per_rank=256,
                        sbuf_free_dim_pad_per_rank=0,
                        sbuf_byte_offset=0)
                if s >= 1:
                    emit_meanA(s - 1)
                if s >= 2:
                    emit_batch_dense(s - 2)

            idtf = cp.tile([P, P], F32, name="idtf")
            nc.sync.dma_start(out=idtf[:], in_=idtf_d.ap()[:, :])
            owt = cp.tile([P, 2, 256], F32, name="owt")
            nc.sync.dma_start(out=owt[:],
                              in_=owt_d.ap()[:, :].rearrange("(i p) f -> p i f", p=P))
            obt = cp.tile([GL, 256], F32, name="obt")
            nc.sync.dma_start(out=obt[:], in_=ob_d.ap()[:, :])

            emit_meanA(S_BATCH - 1)
            for s in range(max(0, S_BATCH - 2), S_BATCH):
                emit_batch_dense(s)
            flush_pool(True)

            # ---- final: normalize + output projection
            den = sb.tile([GL, 4], F32, name="den")
            nc.vector.tensor_scalar_max(out=den[:], in0=pool_ps[:, 256:260],
                                        scalar1=1e-30)
            rden = sb.tile([GL, 4], F32, name="rden")
            nc.vector.reciprocal(out=rden[:], in_=den[:])
            pn = sb.tile([GL, 256], F32, name="pn")
            for hh in range(NUM_HEADS):
                nc.vector.tensor_scalar_mul(out=pn[:, 64 * hh:64 * hh + 64],
                                            in0=pool_ps[:, 64 * hh:64 * hh + 64],
                                            scalar1=rden[:, hh:hh + 1])
            pnT = sb.tile([P, 2, GL], F32, name="pnT")
            for i in range(2):
                ptp = ps.tile([P, GL], F32, name="ptp", tag="ptp", bufs=2)
                nc.tensor.transpose(out=ptp[:], in_=pn[:, i * P:(i + 1) * P],
                                    identity=idtf[:GL, :GL])
                nc.vector.tensor_copy(out=pnT[:, i, :], in_=ptp[:])
            ops_t = ps.tile([GL, 256], F32, name="ops_t", tag="ptp", bufs=2)
            for i in range(2):
                nc.tensor.matmul(out=ops_t[:], lhsT=pnT[:, i, :], rhs=owt[:, i, :],
                                 start=(i == 0), stop=(i == 1))
            osb = sb.tile([GL, 256], F32, name="osb")
            nc.vector.tensor_add(out=osb[:], in0=ops_t[:], in1=obt[:])
            nc.sync.dma_start(out=y_d.ap()[:, :], in_=osb[:])
            if DEBUG_DUMP:
                nc.sync.dma_start(out=phid_d.ap()[:, :], in_=phi[:])
                nc.sync.dma_start(
                    out=tabd_d.ap()[:, :],
                    in_=tabs[0][:].rearrange("p a b c -> p (a b c)"))
                nc.sync.dma_start(
                    out=augd_d.ap()[:, :],
                    in_=augT[:].rearrange("p a b -> p (a b)"))

    nc.finalize()
    return nc


_CACHE = {}


def _get_program(cfg):
    key = tuple(cfg["B_S"])
    if key not in _CACHE:
        _CACHE[key] = _build_program(cfg)
    return _CACHE[key]


def kernel(h, edge_index, edge_attr, batch, num_graphs,
           node_w, node_b, edge_w, edge_b, query, in_w, in_b, out_w, out_b,
           _trace=False, _numpy_only=False):
    import ml_dtypes
    h = np.asarray(h, np.float32)
    edge_attr = np.asarray(edge_attr, np.float32)
    batch_np = np.asarray(batch, np.int64)
    assert int(num_graphs) == G_TOTAL

    cores, cfg = _prep(h, edge_index, edge_attr, batch_np)
    Wbig = _fold_weights(np.asarray(node_w, np.float32), np.asarray(node_b, np.float32),
                         np.asarray(edge_w, np.float32), np.asarray(edge_b, np.float32),
                         np.asarray(query, np.float32), np.asarray(in_w, np.float32),
                         np.asarray(in_b, np.float32))
    out_w32 = np.asarray(out_w, np.float32)
    out_b32 = np.asarray(out_b, np.float32)
    if _numpy_only:
        return _numpy_device_model(cores, cfg, h, edge_attr, batch_np, Wbig,
                                   out_w32, out_b32)

    bf16 = ml_dtypes.bfloat16
    fp8 = ml_dtypes.float8_e4m3 if ES_FP8 else bf16
    tri = (np.arange(P)[:, None] // SLOT <= np.arange(SPC)[None, :]).astype(np.float32)
    idt = np.eye(P)
    iota = np.tile(np.arange(GL, dtype=np.float32)[None, :], (P, 1))
    shared = dict(
        tri=tri.astype(fp8),
        idtf=idt.astype(np.float32),
        iota=iota,
        wb1=Wbig[:128].astype(bf16), wb2=Wbig[128:].astype(bf16),
        owt=np.ascontiguousarray(out_w32.T),
        ob=np.tile(out_b32[None, :], (GL, 1)),
    )
    in_maps = []
    for c, st in enumerate(cores):
        NL = st["NL"]
        stream = np.zeros((cfg["E_PAD"], 64), np.float32)
        stream[st["epos"]] = edge_attr[st["esrc"]]
        hpad = np.zeros((NC_NODES, 128), np.float32)
        hpad[:NL] = h[st["n0"]:st["n1"]][st["perm"]]
        degp = np.zeros(NC_NODES, np.float32)
        degp[:NL] = st["deg"][st["perm"]]
        inv = (1.0 / np.maximum(degp, 1.0)).astype(np.float32)
        blp = np.full(NC_NODES, -1.0, np.float32)
        blp[:NL] = (batch_np[st["n0"]:st["n1"]] - GL * c)[st["perm"]]
        hase = (degp > 0).astype(np.float32)
        hone = np.stack([hase, np.ones(NC_NODES, np.float32)])
        h3 = np.ascontiguousarray(hpad.reshape(NKB, P, 128).transpose(2, 0, 1))
        in_maps.append(dict(
            es=stream.astype(fp8), h=h3.astype(bf16),
            bid=np.ascontiguousarray(blp.reshape(NKB, P).T),
            hi=_wrap_idx(st["hi_row"]),
            inv=np.tile(inv[None, :], (64, 1)).astype(bf16),
            mrep=np.tile(st["mrow"].astype(np.float32)[None, :], (64, 1)).astype(bf16),
            hone=hone.reshape(2, NKB, P).astype(bf16),
            **shared))

    from concourse.bass_utils import run_bass_kernel_spmd
    nc = _get_program(cfg)
    res = run_bass_kernel_spmd(nc, in_maps, core_ids=list(range(CORES)),
                               trace=_trace)
    out = np.concatenate([np.asarray(res.results[c]["y"], np.float32)
                          for c in range(CORES)])
    kernel._last_result = res
    return out.reshape(G_TOTAL, 1, 256)
